# revision 11
# baseline (speedup 1.0000x reference)
"""Trainium2 Bass kernel for nn_GATNodeScorer (GNN message passing).

Strategy (8 NeuronCores, node-partitioned):
  - Host: permute nodes into 160 balanced (core, tile) bins of 128 slots so
    every tile has <= K*128 in-edges; pack edges into 128-edge chunks per
    destination tile; fold attention projections and biases into augmented
    weight matrices.
  - Device, per core (SPMD, one NEFF):
      1. input projection  h = relu(xc @ Wp + bp)    (slab of 2560 nodes)
      2. AllGather H table across 8 cores
      3. relational layer  h1 = h + segsum(h[src] + rel_emb[type]*w)
         via indirect-DMA gather of h[src] rows + one-hot matmul scatter-add
      4. dense x1 = h1 @ [W1 | W1@Asrc | W1@Adst]  -> XA table, AllGather
      5. GAT layer: per edge-chunk gather of [x | a_src] rows, segment
         softmax via exp (no max-subtraction needed; alpha <= ~35) with the
         denominator applied after aggregation, one-hot matmuls accumulate
         numerator and denominator in PSUM
      6. repeat 4-5 for layer 2, then score = h3 @ Wo + bo
  - fp32 end-to-end; matmuls in fp32r (TF32-like, full-rate) where the
    operands are produced by compute ops.

Self-contained: hardcodes all shapes; only needs numpy + the concourse repo
installed at /opt/trn_rl_repo.
"""

import sys

sys.path.insert(0, "/opt/trn_rl_repo")

import heapq

import numpy as np

import concourse.bass as bass
import concourse.bacc as bacc
import concourse.mybir as mybir
import concourse.tile as tile
from concourse.bass_utils import run_bass_kernel_spmd
from concourse.masks import make_identity

# ---- problem constants (hardcoded per contest rules) ----
N, E = 20000, 320000
IN_DIM, CODE_DIM, HIDDEN, HEADS, NREL = 896, 768, 256, 4, 5
CH = HIDDEN // HEADS
CODE_WEIGHT = 3.0
NEG_SLOPE = 0.2

NCORES = 8
P = 128
T = 20  # tiles per core
NTILES = NCORES * T  # 160
NP = T * P  # 2560 padded nodes per core
NPAD = NTILES * P  # 20480
KPROJ = IN_DIM // P  # 7

F32 = mybir.dt.float32
F32R = mybir.dt.float32r
I32 = mybir.dt.int32

XAW = HIDDEN + 2 * HEADS  # 264: [x | a_src | a_dst]
NRELP = 6  # NREL padded even (fp32r matmul requires even free dims)

# ---------------------------------------------------------------------------
# host-side planning
# ---------------------------------------------------------------------------


def _pack_nodes(deg_gat, deg_rel, cap_g, cap_r):
    order = np.argsort(-deg_gat, kind="stable")
    load_g = np.zeros(NTILES, np.int64)
    load_r = np.zeros(NTILES, np.int64)
    count = np.zeros(NTILES, np.int64)
    tile_of = np.full(N, -1, np.int64)
    heap = [(0, t) for t in range(NTILES)]
    heapq.heapify(heap)
    for n in order:
        dg, dr = deg_gat[n], deg_rel[n]
        popped = []
        placed = False
        while heap:
            lg, t = heapq.heappop(heap)
            if lg != load_g[t]:
                continue
            if count[t] < P and load_g[t] + dg <= cap_g and load_r[t] + dr <= cap_r:
                tile_of[n] = t
                load_g[t] += dg
                load_r[t] += dr
                count[t] += 1
                if count[t] < P:
                    heapq.heappush(heap, (load_g[t], t))
                placed = True
                break
            popped.append((lg, t))
        for item in popped:
            heapq.heappush(heap, item)
        if not placed:
            raise RuntimeError("packing failed")
    return tile_of


def _pack_edges(src_pp, dst_pp, K):
    tile_e = dst_pp // P
    order_e = np.argsort(tile_e, kind="stable")
    esrc = np.zeros((NTILES, K * P), np.int32)
    dloc = np.full((NTILES, K * P), P, np.float32)
    eord = np.full((NTILES, K * P), -1, np.int64)
    bounds = np.searchsorted(tile_e[order_e], np.arange(NTILES + 1))
    for t in range(NTILES):
        lo, hi = bounds[t], bounds[t + 1]
        ecnt = hi - lo
        if ecnt > K * P:
            raise RuntimeError(f"tile {t}: {ecnt} edges > {K * P}")
        idxs = order_e[lo:hi]
        esrc[t, :ecnt] = src_pp[idxs]
        dloc[t, :ecnt] = (dst_pp[idxs] - t * P).astype(np.float32)
        eord[t, :ecnt] = idxs
    esrc = np.ascontiguousarray(esrc.reshape(NTILES, K, P).transpose(0, 2, 1))
    dloc = np.ascontiguousarray(dloc.reshape(NTILES, K, P).transpose(0, 2, 1))
    eord = np.ascontiguousarray(eord.reshape(NTILES, K, P).transpose(0, 2, 1))
    return esrc, dloc, eord


def _build_plan(edge_index):
    src = edge_index[0].astype(np.int64)
    dst = edge_index[1].astype(np.int64)
    deg_rel = np.bincount(dst, minlength=N)
    deg_gat = deg_rel + 1
    for K_G, K_R in ((17, 16), (18, 17), (19, 18)):
        try:
            tile_of = _pack_nodes(deg_gat, deg_rel, K_G * P, K_R * P)
            break
        except RuntimeError:
            continue
    else:
        raise RuntimeError("node packing failed at all K")

    perm = np.full(N, -1, np.int64)
    slot_ctr = np.zeros(NTILES, np.int64)
    for n in np.argsort(tile_of, kind="stable"):
        t = tile_of[n]
        perm[n] = t * P + slot_ctr[t]
        slot_ctr[t] += 1

    src_p, dst_p = perm[src], perm[dst]
    esrc_r, dloc_r, eord_r = _pack_edges(src_p, dst_p, K_R)
    loop = perm[np.arange(N)]
    esrc_g, dloc_g, _ = _pack_edges(
        np.concatenate([src_p, loop]), np.concatenate([dst_p, loop]), K_G
    )
    return dict(
        perm=perm,
        K_G=K_G,
        K_R=K_R,
        esrc_r=esrc_r,
        dloc_r=dloc_r,
        eord_r=eord_r,
        esrc_g=esrc_g,
        dloc_g=dloc_g,
    )


def _asrc_mat(att):
    """[HEADS, CH] -> [HIDDEN, HEADS] block matrix so x @ A == (x*att).sum(-1)."""
    A = np.zeros((HIDDEN, HEADS), np.float32)
    for h in range(HEADS):
        A[h * CH : (h + 1) * CH, h] = att[h]
    return A


# ---------------------------------------------------------------------------
# bass program
# ---------------------------------------------------------------------------


def _build_bass(K_R, K_G):
    nc = bacc.Bacc("TRN2", target_bir_lowering=False, debug=False, num_devices=NCORES)

    # ---- external inputs ----
    xtt_in = nc.dram_tensor("xtt", [T, KPROJ, P, P], F32, kind="ExternalInput")
    wp_in = nc.dram_tensor("wp", [KPROJ, P, HIDDEN], F32, kind="ExternalInput")
    bp_in = nc.dram_tensor("bp_row", [1, HIDDEN], F32, kind="ExternalInput")
    w1_in = nc.dram_tensor("w1aug", [2, P, XAW], F32, kind="ExternalInput")
    w2_in = nc.dram_tensor("w2aug", [2, P, XAW], F32, kind="ExternalInput")
    b1w2_in = nc.dram_tensor("b1w2_row", [1, XAW], F32, kind="ExternalInput")
    rel_in = nc.dram_tensor("rel_emb", [NRELP, HIDDEN], F32, kind="ExternalInput")
    worep_in = nc.dram_tensor("wo_rep", [P, HIDDEN], F32, kind="ExternalInput")
    scb_in = nc.dram_tensor("sc_bias", [P, 1], F32, kind="ExternalInput")
    esrc_r_in = nc.dram_tensor("esrc_r", [T, P, K_R], I32, kind="ExternalInput")
    dloc_r_in = nc.dram_tensor("dloc_r", [T, P, K_R], F32, kind="ExternalInput")
    wtyp_r_in = nc.dram_tensor("wtyp_r", [T, P, NRELP * K_R], F32, kind="ExternalInput")
    esrc_g_in = nc.dram_tensor("esrc_g", [T, P, K_G], I32, kind="ExternalInput")
    dloc_g_in = nc.dram_tensor("dloc_g", [T, P, K_G], F32, kind="ExternalInput")

    score_out = nc.dram_tensor("score", [NP], F32, kind="ExternalOutput")

    with tile.TileContext(nc) as tc:
        with (
            tc.tile_pool(name="const", bufs=1) as cpool,
            tc.tile_pool(name="hres", bufs=1) as hpool,
            tc.tile_pool(name="lhsT", bufs=4) as lpool,
            tc.tile_pool(name="edge_idx", bufs=2) as epool,
            tc.tile_pool(name="gather", bufs=3) as gpool,
            tc.tile_pool(name="onehot", bufs=3) as opool,
            tc.tile_pool(name="msg", bufs=3) as mpool,
            tc.tile_pool(name="small", bufs=4) as spool,
            tc.tile_pool(name="ps", bufs=1, space="PSUM") as pspool,
            tc.tile_pool(name="dram", bufs=1, space="DRAM") as dpool,
        ):
            # ---- constants ----
            ident = cpool.tile([P, P], F32)
            make_identity(nc, ident[:])
            iota_row_i = cpool.tile([P, P], I32)
            nc.gpsimd.iota(iota_row_i[:], pattern=[[1, P]], base=0, channel_multiplier=0)
            iota_row = cpool.tile([P, P], F32)
            nc.vector.tensor_copy(iota_row[:], iota_row_i[:])
            iota_col_i = cpool.tile([P, 1], I32)
            nc.gpsimd.iota(iota_col_i[:], pattern=[[0, 1]], base=0, channel_multiplier=1)
            iota_col = cpool.tile([P, 1], F32)
            nc.vector.tensor_copy(iota_col[:], iota_col_i[:])
            ones_row = cpool.tile([1, P], F32)
            nc.vector.memset(ones_row[:], 1.0)
            slope_sb = cpool.tile([P, 1], F32)
            nc.vector.memset(slope_sb[:], NEG_SLOPE)

            # weights resident in SBUF
            wp_sb = cpool.tile([P, KPROJ * HIDDEN], F32)
            for k in range(KPROJ):
                nc.sync.dma_start(
                    wp_sb[:, k * HIDDEN : (k + 1) * HIDDEN], wp_in[k, :, :]
                )
            wp_r = cpool.tile([P, KPROJ * HIDDEN], F32R)
            nc.vector.tensor_copy(wp_r[:], wp_sb[:])

            bp_sb = cpool.tile([1, HIDDEN], F32)
            nc.sync.dma_start(bp_sb[:], bp_in[:, :])

            w_sb = cpool.tile([P, 2 * XAW], F32)  # scratch for rounding
            waug_r = []
            for li, w_in in enumerate((w1_in, w2_in)):
                wr = cpool.tile([P, 2 * XAW], F32R, name=f"w{li}r")
                for k in range(2):
                    nc.sync.dma_start(w_sb[:, k * XAW : (k + 1) * XAW], w_in[k, :, :])
                nc.vector.tensor_copy(wr[:], w_sb[:])
                waug_r.append(wr)

            b1w2_sb = cpool.tile([1, XAW], F32)
            nc.sync.dma_start(b1w2_sb[:], b1w2_in[:, :])
            rel_sb = cpool.tile([NRELP, HIDDEN], F32)
            nc.sync.dma_start(rel_sb[:], rel_in[:, :])
            worep_sb = cpool.tile([P, HIDDEN], F32)
            nc.sync.dma_start(worep_sb[:], worep_in[:, :])
            scb_sb = cpool.tile([P, 1], F32)
            nc.sync.dma_start(scb_sb[:], scb_in[:, :])

            # residual h slabs (two ping-pong slabs of T tiles)
            hA = hpool.tile([P, T * HIDDEN], F32)
            hB = hpool.tile([P, T * HIDDEN], F32)
            adst_all = hpool.tile([P, T * HEADS], F32R)

            # DRAM bounce buffers for collectives
            h_slab = dpool.tile([NP, HIDDEN], F32)
            h_full = dpool.tile([NPAD, HIDDEN], F32, addr_space="Shared")
            xa_slab = dpool.tile([NP, XAW], F32)
            xa_full = dpool.tile([NPAD, XAW], F32, addr_space="Shared")
            xa_slab2 = dpool.tile([NP, XAW], F32)
            xa_full2 = dpool.tile([NPAD, XAW], F32, addr_space="Shared")

            def hcols(t):
                return slice(t * HIDDEN, (t + 1) * HIDDEN)

            # ================= stage 1: input projection =================
            for t in range(T):
                proj_ps = pspool.tile([P, HIDDEN], F32, tag="work", bufs=1)
                for k in range(KPROJ):
                    lx = lpool.tile([P, P], F32, tag="lhsT")
                    nc.sync.dma_start(lx[:], xtt_in[t, k, :, :])
                    lxr = lpool.tile([P, P], F32R, tag="lhsTr")
                    nc.vector.tensor_copy(lxr[:], lx[:])
                    nc.tensor.matmul(
                        out=proj_ps[:],
                        lhsT=lxr[:],
                        rhs=wp_r[:, k * HIDDEN : (k + 1) * HIDDEN],
                        start=(k == 0),
                        stop=False,
                    )
                nc.tensor.matmul(
                    out=proj_ps[:],
                    lhsT=ones_row[:1, :],
                    rhs=bp_sb[:1, :],
                    start=False,
                    stop=True,
                )
                nc.scalar.activation(
                    out=hA[:, hcols(t)],
                    in_=proj_ps[:],
                    func=mybir.ActivationFunctionType.Relu,
                )
                nc.sync.dma_start(h_slab[t * P : (t + 1) * P, :], hA[:, hcols(t)])

            # ================= AllGather H =================
            nc.gpsimd.collective_compute(
                "AllGather",
                mybir.AluOpType.bypass,
                replica_groups=[list(range(NCORES))],
                ins=[h_slab.opt()],
                outs=[h_full.opt()],
            )

            # ================= stage 2: relational layer =================
            for t in range(T):
                esrc_t = epool.tile([P, K_R], I32, tag="esrc")
                nc.sync.dma_start(esrc_t[:], esrc_r_in[t, :, :])
                dloc_t = epool.tile([P, K_R], F32, tag="dloc")
                nc.sync.dma_start(dloc_t[:], dloc_r_in[t, :, :])
                wtyp_t = epool.tile([P, NRELP * K_R], F32, tag="wtyp")
                nc.sync.dma_start(wtyp_t[:], wtyp_r_in[t, :, :])

                out_ps = pspool.tile([P, HIDDEN], F32, tag="out", bufs=2)
                wmat_ps = pspool.tile([P, NRELP], F32, tag="acc4", bufs=2)
                for k in range(K_R):
                    hch = gpool.tile([P, HIDDEN], F32, tag="gather")
                    nc.gpsimd.indirect_dma_start(
                        out=hch[:],
                        out_offset=None,
                        in_=h_full[:, :],
                        in_offset=bass.IndirectOffsetOnAxis(
                            ap=esrc_t[:, k : k + 1], axis=0
                        ),
                    )
                    hch_r = mpool.tile([P, HIDDEN], F32R, tag="msg")
                    nc.scalar.activation(
                        out=hch_r[:],
                        in_=hch[:],
                        func=mybir.ActivationFunctionType.Copy,
                    )
                    oh_r = opool.tile([P, P], F32R, tag="onehot")
                    nc.vector.tensor_tensor(
                        out=oh_r[:],
                        in0=dloc_t[:, k : k + 1].to_broadcast([P, P]),
                        in1=iota_row[:],
                        op=mybir.AluOpType.is_equal,
                    )
                    # numerator accumulate: out += onehot.T @ h[src]
                    nc.tensor.matmul(
                        out=out_ps[:],
                        lhsT=oh_r[:],
                        rhs=hch_r[:],
                        start=(k == 0),
                        stop=False,
                    )
                    # per-type weight accumulate: wmat[p, t] += onehot.T @ wtyp
                    wt_r = spool.tile([P, NRELP], F32R, tag="wt")
                    nc.vector.tensor_copy(
                        wt_r[:], wtyp_t[:, k * NRELP : (k + 1) * NRELP]
                    )
                    nc.tensor.matmul(
                        out=wmat_ps[:],
                        lhsT=oh_r[:],
                        rhs=wt_r[:],
                        start=(k == 0),
                        stop=(k == K_R - 1),
                    )
                # rel contribution: out += wmat.T.T ... need lhsT [t, p]
                wmat_sb = spool.tile([P, NRELP], F32, tag="wmat")
                nc.vector.tensor_copy(wmat_sb[:], wmat_ps[:])
                wmatT_ps = pspool.tile([NRELP, P], F32, tag="tmp", bufs=3)
                nc.tensor.transpose(
                    out=wmatT_ps[:], in_=wmat_sb[:], identity=ident[:]
                )
                wmatT_sb = spool.tile([NRELP, P], F32, tag="wmatT")
                nc.vector.tensor_copy(wmatT_sb[:], wmatT_ps[:])
                nc.tensor.matmul(
                    out=out_ps[:],
                    lhsT=wmatT_sb[:],
                    rhs=rel_sb[:],
                    start=False,
                    stop=True,
                )
                # h1 = h + sum
                nc.vector.tensor_add(hB[:, hcols(t)], out_ps[:], hA[:, hcols(t)])

            # ============ stages 3/4: GAT layers ============
            for layer in range(2):
                hin = hB if layer == 0 else hA
                hout = hA if layer == 0 else hB
                wr = waug_r[layer]
                slab = xa_slab if layer == 0 else xa_slab2
                full = xa_full if layer == 0 else xa_full2

                # ---- dense: x = h @ Waug (+ b-fold for layer 1) ----
                for t in range(T):
                    x_ps = pspool.tile([P, XAW], F32, tag="work", bufs=1)
                    for half in range(2):
                        tr_ps = pspool.tile([P, P], F32, tag="tmp", bufs=3)
                        nc.tensor.transpose(
                            out=tr_ps[:],
                            in_=hin[:, t * HIDDEN + half * P : t * HIDDEN + (half + 1) * P],
                            identity=ident[:],
                        )
                        ht_r = lpool.tile([P, P], F32R, tag="lhsTr")
                        nc.vector.tensor_copy(ht_r[:], tr_ps[:])
                        nc.tensor.matmul(
                            out=x_ps[:],
                            lhsT=ht_r[:],
                            rhs=wr[:, half * XAW : (half + 1) * XAW],
                            start=(half == 0),
                            stop=(half == 1 and layer == 0),
                        )
                    if layer == 1:
                        # fold h2 = gat1_out + b1 into x2 = h2 @ W2aug
                        nc.tensor.matmul(
                            out=x_ps[:],
                            lhsT=ones_row[:1, :],
                            rhs=b1w2_sb[:1, :],
                            start=False,
                            stop=True,
                        )
                    xa_sb = gpool.tile([P, XAW], F32, tag="xa_sb")
                    nc.vector.tensor_copy(xa_sb[:], x_ps[:])
                    nc.sync.dma_start(slab[t * P : (t + 1) * P, :], xa_sb[:])
                    # a_dst columns resident (rounded)
                    nc.vector.tensor_copy(
                        adst_all[:, t * HEADS : (t + 1) * HEADS],
                        x_ps[:, HIDDEN + HEADS : HIDDEN + 2 * HEADS],
                    )

                nc.gpsimd.collective_compute(
                    "AllGather",
                    mybir.AluOpType.bypass,
                    replica_groups=[list(range(NCORES))],
                    ins=[slab.opt()],
                    outs=[full.opt()],
                )

                # ---- edge stage ----
                for t in range(T):
                    esrc_t = epool.tile([P, K_G], I32, tag="esrc")
                    nc.sync.dma_start(esrc_t[:], esrc_g_in[t, :, :])
                    dloc_t = epool.tile([P, K_G], F32, tag="dloc")
                    nc.sync.dma_start(dloc_t[:], dloc_g_in[t, :, :])

                    out_ps = pspool.tile([P, HIDDEN], F32, tag="out", bufs=2)
                    den_ps = pspool.tile([P, HEADS], F32, tag="acc4", bufs=2)
                    for k in range(K_G):
                        xa = gpool.tile([P, XAW], F32, tag="gather")
                        nc.gpsimd.indirect_dma_start(
                            out=xa[:],
                            out_offset=None,
                            in_=full[:, :],
                            in_offset=bass.IndirectOffsetOnAxis(
                                ap=esrc_t[:, k : k + 1], axis=0
                            ),
                        )
                        # one-hot [e, p]
                        oh_r = opool.tile([P, P], F32R, tag="onehot")
                        nc.vector.tensor_tensor(
                            out=oh_r[:],
                            in0=dloc_t[:, k : k + 1].to_broadcast([P, P]),
                            in1=iota_row[:],
                            op=mybir.AluOpType.is_equal,
                        )
                        # transposed one-hot [p, e] via PE transpose of column bcast
                        row_ps = pspool.tile([P, P], F32, tag="tmp", bufs=3)
                        nc.tensor.transpose(
                            out=row_ps[:],
                            in_=dloc_t[:, k : k + 1].to_broadcast([P, P]),
                            identity=ident[:],
                        )
                        ohT_r = opool.tile([P, P], F32R, tag="onehotT")
                        nc.vector.tensor_tensor(
                            out=ohT_r[:],
                            in0=iota_col[:].to_broadcast([P, P]),
                            in1=row_ps[:],
                            op=mybir.AluOpType.is_equal,
                        )
                        # edge a_dst [e, 4]
                        ea_ps = pspool.tile([P, HEADS], F32, tag="tmp", bufs=3)
                        nc.tensor.matmul(
                            out=ea_ps[:],
                            lhsT=ohT_r[:],
                            rhs=adst_all[:, t * HEADS : (t + 1) * HEADS],
                            start=True,
                            stop=True,
                        )
                        # alpha = a_src + edge_a_dst ; lrelu; exp
                        alpha_sb = spool.tile([P, HEADS], F32, tag="alpha")
                        nc.vector.tensor_add(
                            alpha_sb[:], xa[:, HIDDEN : HIDDEN + HEADS], ea_ps[:]
                        )
                        # leaky relu via Prelu (ACT Lrelu ignores alpha; Prelu
                        # with a per-partition alpha AP is exact)
                        lr_sb = spool.tile([P, HEADS], F32, tag="lr")
                        nc.scalar.activation(
                            out=lr_sb[:],
                            in_=alpha_sb[:],
                            func=mybir.ActivationFunctionType.Prelu,
                            alpha=slope_sb[:],
                        )
                        ex_r = spool.tile([P, HEADS], F32R, tag="ex")
                        nc.scalar.activation(
                            out=ex_r[:],
                            in_=lr_sb[:],
                            func=mybir.ActivationFunctionType.Exp,
                        )
                        # denominator accumulate
                        nc.tensor.matmul(
                            out=den_ps[:],
                            lhsT=oh_r[:],
                            rhs=ex_r[:],
                            start=(k == 0),
                            stop=(k == K_G - 1),
                        )
                        # msg = x * ex (per head), then accumulate
                        msg_r = mpool.tile([P, HIDDEN], F32R, tag="msg")
                        nc.vector.tensor_tensor(
                            out=msg_r[:].rearrange("p (h c) -> p h c", h=HEADS),
                            in0=xa[:, :HIDDEN].rearrange("p (h c) -> p h c", h=HEADS),
                            in1=ex_r[:].unsqueeze(-1).to_broadcast([P, HEADS, CH]),
                            op=mybir.AluOpType.mult,
                        )
                        nc.tensor.matmul(
                            out=out_ps[:],
                            lhsT=oh_r[:],
                            rhs=msg_r[:],
                            start=(k == 0),
                            stop=(k == K_G - 1),
                        )
                    # normalize: h_next = out / denom
                    den_sb = spool.tile([P, HEADS], F32, tag="den")
                    nc.vector.tensor_scalar_add(den_sb[:], den_ps[:], 1e-30)
                    dinv_sb = spool.tile([P, HEADS], F32, tag="dinv")
                    nc.vector.reciprocal(dinv_sb[:], den_sb[:])
                    nc.vector.tensor_tensor(
                        out=hout[:, hcols(t)].rearrange("p (h c) -> p h c", h=HEADS),
                        in0=out_ps[:].rearrange("p (h c) -> p h c", h=HEADS),
                        in1=dinv_sb[:].unsqueeze(-1).to_broadcast([P, HEADS, CH]),
                        op=mybir.AluOpType.mult,
                    )

            # ================= stage 5: score =================
            for t in range(T):
                prod = gpool.tile([P, HIDDEN], F32, tag="xa_sb")
                nc.vector.tensor_mul(prod[:], hB[:, hcols(t)], worep_sb[:])
                red = spool.tile([P, 1], F32, tag="red")
                nc.vector.tensor_reduce(
                    out=red[:],
                    in_=prod[:],
                    axis=mybir.AxisListType.X,
                    op=mybir.AluOpType.add,
                )
                sc = spool.tile([P, 1], F32, tag="sc")
                nc.vector.tensor_add(sc[:], red[:], scb_sb[:])
                nc.sync.dma_start(score_out[t * P : (t + 1) * P], sc[:])

    nc.compile()
    return nc


# ---------------------------------------------------------------------------
# entry point
# ---------------------------------------------------------------------------

_CACHE = {}


def prepare(inputs, plan):
    """Build (in_maps, nc, perm) from the full input dict + plan."""
    x = np.asarray(inputs["x"], np.float32)
    edge_type = np.asarray(inputs["edge_type"], np.int32)
    edge_weight = np.asarray(inputs["edge_weight"], np.float32)
    rel_emb = np.asarray(inputs["rel_emb"], np.float32)
    Wp = np.asarray(inputs["Wp"], np.float32)
    bp = np.asarray(inputs["bp"], np.float32)
    W1 = np.asarray(inputs["W1"], np.float32)
    W2 = np.asarray(inputs["W2"], np.float32)
    att_src1 = np.asarray(inputs["att_src1"], np.float32)
    att_dst1 = np.asarray(inputs["att_dst1"], np.float32)
    att_src2 = np.asarray(inputs["att_src2"], np.float32)
    att_dst2 = np.asarray(inputs["att_dst2"], np.float32)
    b1 = np.asarray(inputs["b1"], np.float32)
    b2 = np.asarray(inputs["b2"], np.float32)
    Wo = np.asarray(inputs["Wo"], np.float32)
    bo = np.asarray(inputs["bo"], np.float32)

    perm = plan["perm"]
    K_R, K_G = plan["K_R"], plan["K_G"]

    # ---- per-core dense inputs ----
    xr = np.concatenate([x[:, CODE_DIM:], CODE_WEIGHT * x[:, :CODE_DIM]], axis=1)
    xpad = np.zeros((NPAD, IN_DIM), np.float32)
    xpad[perm] = xr
    # [NCORES, T, KPROJ, P(feat), P(node)]
    xtt = (
        xpad.reshape(NCORES, T, P, KPROJ, P)
        .transpose(0, 1, 3, 4, 2)
        .copy()
    )

    w1aug = np.concatenate(
        [W1, W1 @ _asrc_mat(att_src1), W1 @ _asrc_mat(att_dst1)], axis=1
    )
    w2aug = np.concatenate(
        [W2, W2 @ _asrc_mat(att_src2), W2 @ _asrc_mat(att_dst2)], axis=1
    )
    b1w2 = (b1 @ w2aug).reshape(1, XAW).astype(np.float32)
    sc_bias = float(b2 @ Wo[:, 0] + bo[0])

    # ---- per-edge rel wtype rows: w_e * onehot5(type_e) ----
    eord_r = plan["eord_r"]  # [NTILES, P, K_R]
    wtyp = np.zeros((NTILES, P, K_R, NRELP), np.float32)
    valid = eord_r >= 0
    ew = np.where(valid, edge_weight[np.clip(eord_r, 0, E - 1)], 0.0).astype(np.float32)
    et = np.where(valid, edge_type[np.clip(eord_r, 0, E - 1)], 0)
    ii, jj, kk = np.nonzero(valid)
    wtyp[ii, jj, kk, et[ii, jj, kk]] = ew[ii, jj, kk]
    wtyp = wtyp.reshape(NTILES, P, K_R * NRELP)

    key = (K_R, K_G)
    if key not in _CACHE:
        _CACHE[key] = _build_bass(K_R, K_G)
    nc = _CACHE[key]

    common = dict(
        wp=np.ascontiguousarray(Wp.reshape(KPROJ, P, HIDDEN)),
        bp_row=bp.reshape(1, HIDDEN),
        w1aug=np.ascontiguousarray(w1aug.reshape(2, P, XAW)),
        w2aug=np.ascontiguousarray(w2aug.reshape(2, P, XAW)),
        b1w2_row=b1w2,
        rel_emb=np.concatenate([rel_emb, np.zeros((NRELP - NREL, HIDDEN), np.float32)]),
        wo_rep=np.ascontiguousarray(np.broadcast_to(Wo[:, 0], (P, HIDDEN))),
        sc_bias=np.full((P, 1), sc_bias, np.float32),
    )
    in_maps = []
    for c in range(NCORES):
        ts = slice(c * T, (c + 1) * T)
        in_maps.append(
            dict(
                common,
                xtt=xtt[c],
                esrc_r=plan["esrc_r"][ts],
                dloc_r=plan["dloc_r"][ts],
                wtyp_r=np.ascontiguousarray(wtyp[ts]),
                esrc_g=plan["esrc_g"][ts],
                dloc_g=plan["dloc_g"][ts],
            )
        )
    return in_maps, nc, perm


def kernel(x, edge_index, **rest):
    inputs = dict(rest, x=x, edge_index=edge_index)
    edge_index = np.asarray(edge_index, np.int32)
    plan = _build_plan(edge_index)
    in_maps, nc, perm = prepare(inputs, plan)

    import os

    trace = bool(os.environ.get("GAT_TRACE"))
    res = run_bass_kernel_spmd(
        nc, in_maps, core_ids=list(range(NCORES)), trace=trace
    )
    global _LAST_RESULT
    _LAST_RESULT = res
    scores_pad = np.concatenate([r["score"] for r in res.results])
    return scores_pad[perm].astype(np.float32)


_LAST_RESULT = None


# revision 17
# speedup vs baseline: 3.7558x; 3.7558x over previous
"""Trainium2 Bass kernel for nn_GATNodeScorer (GNN message passing).

Strategy (8 NeuronCores, node-partitioned):
  - Host: permute nodes into 160 balanced (core, tile) bins of 128 slots so
    every tile has <= K*128 in-edges; pack edges into 128-edge chunks per
    destination tile; fold attention projections and biases into augmented
    weight matrices.
  - Device, per core (SPMD, one NEFF):
      1. input projection  h = relu(xc @ Wp + bp)    (slab of 2560 nodes)
      2. AllGather H table across 8 cores
      3. relational layer  h1 = h + segsum(h[src] + rel_emb[type]*w)
         via indirect-DMA gather of h[src] rows + one-hot matmul scatter-add
      4. dense x1 = h1 @ [W1 | W1@Asrc | W1@Adst]  -> XA table, AllGather
      5. GAT layer: per edge-chunk gather of [x | a_src] rows, segment
         softmax via exp (no max-subtraction needed; alpha <= ~35) with the
         denominator applied after aggregation, one-hot matmuls accumulate
         numerator and denominator in PSUM
      6. repeat 4-5 for layer 2, then score = h3 @ Wo + bo
  - fp32 end-to-end; matmuls in fp32r (TF32-like, full-rate) where the
    operands are produced by compute ops.

Self-contained: hardcodes all shapes; only needs numpy + the concourse repo
installed at /opt/trn_rl_repo.
"""

import sys

sys.path.insert(0, "/opt/trn_rl_repo")

import heapq

import numpy as np

import concourse.bass as bass
import concourse.bacc as bacc
import concourse.mybir as mybir
import concourse.tile as tile
from concourse.bass_utils import run_bass_kernel_spmd
from concourse.masks import make_identity

# ---- problem constants (hardcoded per contest rules) ----
N, E = 20000, 320000
IN_DIM, CODE_DIM, HIDDEN, HEADS, NREL = 896, 768, 256, 4, 5
CH = HIDDEN // HEADS
CODE_WEIGHT = 3.0
NEG_SLOPE = 0.2

NCORES = 8
P = 128
T = 20  # tiles per core
NTILES = NCORES * T  # 160
NP = T * P  # 2560 padded nodes per core
NPAD = NTILES * P  # 20480
KPROJ = IN_DIM // P  # 7

F32 = mybir.dt.float32
F32R = mybir.dt.float32r
I32 = mybir.dt.int32

XAW = HIDDEN + 2 * HEADS  # 264: [x | a_src | a_dst]
NRELP = 6  # NREL padded even (fp32r matmul requires even free dims)

# ---------------------------------------------------------------------------
# host-side planning
# ---------------------------------------------------------------------------


def _pack_nodes(deg_gat, deg_rel, cap_g, cap_r):
    order = np.argsort(-deg_gat, kind="stable")
    load_g = np.zeros(NTILES, np.int64)
    load_r = np.zeros(NTILES, np.int64)
    count = np.zeros(NTILES, np.int64)
    tile_of = np.full(N, -1, np.int64)
    heap = [(0, t) for t in range(NTILES)]
    heapq.heapify(heap)
    for n in order:
        dg, dr = deg_gat[n], deg_rel[n]
        popped = []
        placed = False
        while heap:
            lg, t = heapq.heappop(heap)
            if lg != load_g[t]:
                continue
            if count[t] < P and load_g[t] + dg <= cap_g and load_r[t] + dr <= cap_r:
                tile_of[n] = t
                load_g[t] += dg
                load_r[t] += dr
                count[t] += 1
                if count[t] < P:
                    heapq.heappush(heap, (load_g[t], t))
                placed = True
                break
            popped.append((lg, t))
        for item in popped:
            heapq.heappush(heap, item)
        if not placed:
            raise RuntimeError("packing failed")
    return tile_of


def _pack_edges(src_pp, dst_pp, K):
    tile_e = dst_pp // P
    order_e = np.argsort(tile_e, kind="stable")
    esrc = np.zeros((NTILES, K * P), np.int32)
    dloc = np.full((NTILES, K * P), P, np.float32)
    eord = np.full((NTILES, K * P), -1, np.int64)
    bounds = np.searchsorted(tile_e[order_e], np.arange(NTILES + 1))
    for t in range(NTILES):
        lo, hi = bounds[t], bounds[t + 1]
        ecnt = hi - lo
        if ecnt > K * P:
            raise RuntimeError(f"tile {t}: {ecnt} edges > {K * P}")
        idxs = order_e[lo:hi]
        esrc[t, :ecnt] = src_pp[idxs]
        dloc[t, :ecnt] = (dst_pp[idxs] - t * P).astype(np.float32)
        eord[t, :ecnt] = idxs
    esrc = np.ascontiguousarray(esrc.reshape(NTILES, K, P).transpose(0, 2, 1))
    dloc = np.ascontiguousarray(dloc.reshape(NTILES, K, P).transpose(0, 2, 1))
    eord = np.ascontiguousarray(eord.reshape(NTILES, K, P).transpose(0, 2, 1))
    return esrc, dloc, eord


def _build_plan(edge_index):
    src = edge_index[0].astype(np.int64)
    dst = edge_index[1].astype(np.int64)
    deg_rel = np.bincount(dst, minlength=N)
    deg_gat = deg_rel + 1
    for K_G, K_R in ((17, 16), (18, 17), (19, 18)):
        try:
            tile_of = _pack_nodes(deg_gat, deg_rel, K_G * P, K_R * P)
            break
        except RuntimeError:
            continue
    else:
        raise RuntimeError("node packing failed at all K")

    perm = np.full(N, -1, np.int64)
    slot_ctr = np.zeros(NTILES, np.int64)
    for n in np.argsort(tile_of, kind="stable"):
        t = tile_of[n]
        perm[n] = t * P + slot_ctr[t]
        slot_ctr[t] += 1

    src_p, dst_p = perm[src], perm[dst]
    esrc_r, dloc_r, eord_r = _pack_edges(src_p, dst_p, K_R)
    loop = perm[np.arange(N)]
    esrc_g, dloc_g, _ = _pack_edges(
        np.concatenate([src_p, loop]), np.concatenate([dst_p, loop]), K_G
    )
    return dict(
        perm=perm,
        K_G=K_G,
        K_R=K_R,
        esrc_r=esrc_r,
        dloc_r=dloc_r,
        eord_r=eord_r,
        esrc_g=esrc_g,
        dloc_g=dloc_g,
    )


def _asrc_mat(att):
    """[HEADS, CH] -> [HIDDEN, HEADS] block matrix so x @ A == (x*att).sum(-1)."""
    A = np.zeros((HIDDEN, HEADS), np.float32)
    for h in range(HEADS):
        A[h * CH : (h + 1) * CH, h] = att[h]
    return A


# ---------------------------------------------------------------------------
# bass program
# ---------------------------------------------------------------------------


def _build_bass(K_R, K_G, probe=None):
    probe = probe or {}
    nc = bacc.Bacc("TRN2", target_bir_lowering=False, debug=False, num_devices=NCORES)

    # ---- external inputs ----
    xtt_in = nc.dram_tensor("xtt", [T, KPROJ, P, P], F32, kind="ExternalInput")
    wp_in = nc.dram_tensor("wp", [KPROJ, P, HIDDEN], F32, kind="ExternalInput")
    bp_in = nc.dram_tensor("bp_row", [1, HIDDEN], F32, kind="ExternalInput")
    w1_in = nc.dram_tensor("w1aug", [2, P, XAW], F32, kind="ExternalInput")
    w2_in = nc.dram_tensor("w2aug", [2, P, XAW], F32, kind="ExternalInput")
    b1w2_in = nc.dram_tensor("b1w2_row", [1, XAW], F32, kind="ExternalInput")
    rel_in = nc.dram_tensor("rel_emb", [NRELP, HIDDEN], F32, kind="ExternalInput")
    worep_in = nc.dram_tensor("wo_rep", [P, HIDDEN], F32, kind="ExternalInput")
    scb_in = nc.dram_tensor("sc_bias", [P, 1], F32, kind="ExternalInput")
    esrc_r_in = nc.dram_tensor("esrc_r", [T, P, K_R], I32, kind="ExternalInput")
    dloc_r_in = nc.dram_tensor("dloc_r", [T, P, K_R], F32, kind="ExternalInput")
    wtyp_r_in = nc.dram_tensor("wtyp_r", [T, P, NRELP * K_R], F32, kind="ExternalInput")
    esrc_g_in = nc.dram_tensor("esrc_g", [T, P, K_G], I32, kind="ExternalInput")
    dloc_g_in = nc.dram_tensor("dloc_g", [T, P, K_G], F32, kind="ExternalInput")

    score_out = nc.dram_tensor("score", [NP], F32, kind="ExternalOutput")

    with tile.TileContext(nc) as tc:
        with (
            tc.tile_pool(name="const", bufs=1) as cpool,
            tc.tile_pool(name="hres", bufs=1) as hpool,
            tc.tile_pool(name="lhsT", bufs=4) as lpool,
            tc.tile_pool(name="edge_idx", bufs=2) as epool,
            tc.tile_pool(name="gather", bufs=2) as gpool,
            tc.tile_pool(name="onehot", bufs=2) as opool,
            tc.tile_pool(name="msg", bufs=2) as mpool,
            tc.tile_pool(name="small", bufs=4) as spool,
            tc.tile_pool(name="ps", bufs=1, space="PSUM") as pspool,
            tc.tile_pool(name="dram", bufs=1, space="DRAM") as dpool,
        ):
            # ---- constants ----
            ident = cpool.tile([P, P], F32)
            make_identity(nc, ident[:])
            iota_row_i = cpool.tile([P, P], I32)
            nc.gpsimd.iota(iota_row_i[:], pattern=[[1, P]], base=0, channel_multiplier=0)
            iota_row = cpool.tile([P, P], F32)
            nc.vector.tensor_copy(iota_row[:], iota_row_i[:])
            iota_col_i = cpool.tile([P, 1], I32)
            nc.gpsimd.iota(iota_col_i[:], pattern=[[0, 1]], base=0, channel_multiplier=1)
            iota_col = cpool.tile([P, 1], F32)
            nc.vector.tensor_copy(iota_col[:], iota_col_i[:])
            ones_row = cpool.tile([1, P], F32)
            nc.vector.memset(ones_row[:], 1.0)
            slope_sb = cpool.tile([P, 1], F32)
            nc.vector.memset(slope_sb[:], NEG_SLOPE)

            # weights resident in SBUF
            wp_sb = cpool.tile([P, KPROJ * HIDDEN], F32)
            for k in range(KPROJ):
                nc.sync.dma_start(
                    wp_sb[:, k * HIDDEN : (k + 1) * HIDDEN], wp_in[k, :, :]
                )
            wp_r = cpool.tile([P, KPROJ * HIDDEN], F32R)
            nc.vector.tensor_copy(wp_r[:], wp_sb[:])

            bp_sb = cpool.tile([1, HIDDEN], F32)
            nc.sync.dma_start(bp_sb[:], bp_in[:, :])

            w_sb = cpool.tile([P, 2 * XAW], F32)  # scratch for rounding
            waug_r = []
            for li, w_in in enumerate((w1_in, w2_in)):
                wr = cpool.tile([P, 2 * XAW], F32R, name=f"w{li}r")
                for k in range(2):
                    nc.sync.dma_start(w_sb[:, k * XAW : (k + 1) * XAW], w_in[k, :, :])
                nc.vector.tensor_copy(wr[:], w_sb[:])
                waug_r.append(wr)

            b1w2_sb = cpool.tile([1, XAW], F32)
            nc.sync.dma_start(b1w2_sb[:], b1w2_in[:, :])
            rel_sb = cpool.tile([NRELP, HIDDEN], F32)
            nc.sync.dma_start(rel_sb[:], rel_in[:, :])
            worep_sb = cpool.tile([P, HIDDEN], F32)
            nc.sync.dma_start(worep_sb[:], worep_in[:, :])
            scb_sb = cpool.tile([P, 1], F32)
            nc.sync.dma_start(scb_sb[:], scb_in[:, :])

            # residual h slabs (two ping-pong slabs of T tiles)
            hA = hpool.tile([P, T * HIDDEN], F32)
            hB = hpool.tile([P, T * HIDDEN], F32)
            adst_all = hpool.tile([P, T * HEADS], F32R)

            # DRAM bounce buffers for collectives
            h_slab = dpool.tile([NP, HIDDEN], F32)
            h_full = dpool.tile([NPAD, HIDDEN], F32, addr_space="Shared")
            xa_slab = dpool.tile([NP, XAW], F32)
            xa_full = dpool.tile([NPAD, XAW], F32, addr_space="Shared")
            xa_slab2 = dpool.tile([NP, XAW], F32)
            xa_full2 = dpool.tile([NPAD, XAW], F32, addr_space="Shared")

            def hcols(t):
                return slice(t * HIDDEN, (t + 1) * HIDDEN)

            # ================= stage 1: input projection =================
            for t in range(T):
                proj_ps = pspool.tile([P, HIDDEN], F32, tag="work", bufs=1)
                for k in range(KPROJ):
                    lx = lpool.tile([P, P], F32, tag="lhsT")
                    nc.sync.dma_start(lx[:], xtt_in[t, k, :, :])
                    lxr = lpool.tile([P, P], F32R, tag="lhsTr")
                    nc.vector.tensor_copy(lxr[:], lx[:])
                    nc.tensor.matmul(
                        out=proj_ps[:],
                        lhsT=lxr[:],
                        rhs=wp_r[:, k * HIDDEN : (k + 1) * HIDDEN],
                        start=(k == 0),
                        stop=False,
                    )
                nc.tensor.matmul(
                    out=proj_ps[:],
                    lhsT=ones_row[:1, :],
                    rhs=bp_sb[:1, :],
                    start=False,
                    stop=True,
                )
                nc.scalar.activation(
                    out=hA[:, hcols(t)],
                    in_=proj_ps[:],
                    func=mybir.ActivationFunctionType.Relu,
                )
                nc.sync.dma_start(h_slab[t * P : (t + 1) * P, :], hA[:, hcols(t)])

            if probe.get("stop_after") == "proj":
                return nc
            # ================= AllGather H =================
            if probe.get("no_collective"):
                nc.sync.dma_start(h_full[0:NP, :], h_slab[:, :])
            else:
                nc.gpsimd.collective_compute(
                    "AllGather",
                    mybir.AluOpType.bypass,
                    replica_groups=[list(range(NCORES))],
                    ins=[h_slab.opt()],
                    outs=[h_full.opt()],
                )

            # ================= stage 2: relational layer =================
            for t in range(T):
                esrc_t = epool.tile([P, K_R], I32, tag="esrc")
                nc.sync.dma_start(esrc_t[:], esrc_r_in[t, :, :])
                dloc_t = epool.tile([P, K_R], F32, tag="dloc")
                nc.sync.dma_start(dloc_t[:], dloc_r_in[t, :, :])
                wtyp_t = epool.tile([P, NRELP * K_R], F32, tag="wtyp")
                nc.sync.dma_start(wtyp_t[:], wtyp_r_in[t, :, :])

                out_ps = pspool.tile([P, HIDDEN], F32, tag="out", bufs=2)
                wmat_ps = pspool.tile([P, NRELP], F32, tag="acc4", bufs=2)
                # batched gather of all K_R chunks for this tile
                hch_all = gpool.tile([P, K_R * HIDDEN], F32, tag="gather")
                for k in range(K_R):
                    nc.gpsimd.indirect_dma_start(
                        out=hch_all[:, k * HIDDEN : (k + 1) * HIDDEN],
                        out_offset=None,
                        in_=h_full[:, :],
                        in_offset=bass.IndirectOffsetOnAxis(
                            ap=esrc_t[:, k : k + 1], axis=0
                        ),
                    )
                # one fp32r rounding copy for the whole tile (ACT is idle here)
                hch_r = mpool.tile([P, K_R * HIDDEN], F32R, tag="msg")
                nc.scalar.activation(
                    out=hch_r[:],
                    in_=hch_all[:],
                    func=mybir.ActivationFunctionType.Copy,
                )
                # all one-hots in one DVE op
                oh_all = opool.tile([P, K_R * P], F32R, tag="onehot")
                nc.vector.tensor_tensor(
                    out=oh_all[:].rearrange("p (k e) -> p k e", k=K_R),
                    in0=dloc_t[:].unsqueeze(-1).to_broadcast([P, K_R, P]),
                    in1=iota_row[:].unsqueeze(1).to_broadcast([P, K_R, P]),
                    op=mybir.AluOpType.is_equal,
                )
                wt_r = spool.tile([P, NRELP * K_R], F32R, tag="wt")
                nc.vector.tensor_copy(wt_r[:], wtyp_t[:])
                for k in range(K_R):
                    oh_k = oh_all[:, k * P : (k + 1) * P]
                    # numerator accumulate: out += onehot.T @ h[src]
                    nc.tensor.matmul(
                        out=out_ps[:],
                        lhsT=oh_k,
                        rhs=hch_r[:, k * HIDDEN : (k + 1) * HIDDEN],
                        start=(k == 0),
                        stop=False,
                    )
                    # per-type weight accumulate: wmat[p, t] += onehot.T @ wtyp
                    nc.tensor.matmul(
                        out=wmat_ps[:],
                        lhsT=oh_k,
                        rhs=wt_r[:, k * NRELP : (k + 1) * NRELP],
                        start=(k == 0),
                        stop=(k == K_R - 1),
                    )
                # rel contribution: out += wmat.T.T ... need lhsT [t, p]
                wmat_sb = spool.tile([P, NRELP], F32, tag="wmat")
                nc.vector.tensor_copy(wmat_sb[:], wmat_ps[:])
                wmatT_ps = pspool.tile([NRELP, P], F32, tag="tmp", bufs=2)
                nc.tensor.transpose(
                    out=wmatT_ps[:], in_=wmat_sb[:], identity=ident[:]
                )
                wmatT_sb = spool.tile([NRELP, P], F32, tag="wmatT")
                nc.vector.tensor_copy(wmatT_sb[:], wmatT_ps[:])
                nc.tensor.matmul(
                    out=out_ps[:],
                    lhsT=wmatT_sb[:],
                    rhs=rel_sb[:],
                    start=False,
                    stop=True,
                )
                # h1 = h + sum
                nc.vector.tensor_add(hB[:, hcols(t)], out_ps[:], hA[:, hcols(t)])

            if probe.get("stop_after") == "rel":
                return nc
            # ============ stages 3/4: GAT layers ============
            for layer in range(2):
                hin = hB if layer == 0 else hA
                hout = hA if layer == 0 else hB
                wr = waug_r[layer]
                slab = xa_slab if layer == 0 else xa_slab2
                full = xa_full if layer == 0 else xa_full2

                # ---- dense: x = h @ Waug (+ b-fold for layer 1) ----
                for t in range(T):
                    x_ps = pspool.tile([P, XAW], F32, tag="work", bufs=1)
                    for half in range(2):
                        tr_ps = pspool.tile([P, P], F32, tag="tmp", bufs=2)
                        nc.tensor.transpose(
                            out=tr_ps[:],
                            in_=hin[:, t * HIDDEN + half * P : t * HIDDEN + (half + 1) * P],
                            identity=ident[:],
                        )
                        ht_r = lpool.tile([P, P], F32R, tag="lhsTr")
                        nc.vector.tensor_copy(ht_r[:], tr_ps[:])
                        nc.tensor.matmul(
                            out=x_ps[:],
                            lhsT=ht_r[:],
                            rhs=wr[:, half * XAW : (half + 1) * XAW],
                            start=(half == 0),
                            stop=(half == 1 and layer == 0),
                        )
                    if layer == 1:
                        # fold h2 = gat1_out + b1 into x2 = h2 @ W2aug
                        nc.tensor.matmul(
                            out=x_ps[:],
                            lhsT=ones_row[:1, :],
                            rhs=b1w2_sb[:1, :],
                            start=False,
                            stop=True,
                        )
                    xa_sb = gpool.tile([P, XAW], F32, tag="xa_sb")
                    nc.vector.tensor_copy(xa_sb[:], x_ps[:])
                    nc.sync.dma_start(slab[t * P : (t + 1) * P, :], xa_sb[:])
                    # a_dst columns resident (rounded)
                    nc.vector.tensor_copy(
                        adst_all[:, t * HEADS : (t + 1) * HEADS],
                        x_ps[:, HIDDEN + HEADS : HIDDEN + 2 * HEADS],
                    )

                if probe.get("no_collective"):
                    nc.sync.dma_start(full[0:NP, :], slab[:, :])
                else:
                    nc.gpsimd.collective_compute(
                        "AllGather",
                        mybir.AluOpType.bypass,
                        replica_groups=[list(range(NCORES))],
                        ins=[slab.opt()],
                        outs=[full.opt()],
                    )

                # ---- edge stage ----
                if probe.get("stop_after") == f"dense{layer + 1}":
                    return nc
                for t in range(T):
                    esrc_t = epool.tile([P, K_G], I32, tag="esrc")
                    nc.sync.dma_start(esrc_t[:], esrc_g_in[t, :, :])
                    dloc_t = epool.tile([P, K_G], F32, tag="dloc")
                    nc.sync.dma_start(dloc_t[:], dloc_g_in[t, :, :])

                    out_ps = pspool.tile([P, HIDDEN], F32, tag="out", bufs=2)
                    den_ps = pspool.tile([P, HEADS], F32, tag="acc4", bufs=2)
                    # batched gather for the whole tile
                    xa_all = gpool.tile([P, K_G * XAW], F32, tag="gather")
                    for k in range(K_G):
                        nc.gpsimd.indirect_dma_start(
                            out=xa_all[:, k * XAW : (k + 1) * XAW],
                            out_offset=None,
                            in_=full[:, :],
                            in_offset=bass.IndirectOffsetOnAxis(
                                ap=esrc_t[:, k : k + 1], axis=0
                            ),
                        )
                    xa_v = xa_all[:].rearrange("p (k w) -> p k w", k=K_G)
                    # all one-hots [e, p] in one DVE op
                    oh_all = opool.tile([P, K_G * P], F32R, tag="onehot")
                    nc.vector.tensor_tensor(
                        out=oh_all[:].rearrange("p (k e) -> p k e", k=K_G),
                        in0=dloc_t[:].unsqueeze(-1).to_broadcast([P, K_G, P]),
                        in1=iota_row[:].unsqueeze(1).to_broadcast([P, K_G, P]),
                        op=mybir.AluOpType.is_equal,
                    )
                    # per-chunk transposed one-hots + edge-a_dst into one PSUM row
                    ea_ps = pspool.tile([P, K_G * HEADS], F32, tag="ea", bufs=1)
                    for k in range(K_G):
                        row_ps = pspool.tile([P, P], F32, tag="tmp", bufs=2)
                        nc.tensor.transpose(
                            out=row_ps[:],
                            in_=dloc_t[:, k : k + 1].to_broadcast([P, P]),
                            identity=ident[:],
                        )
                        ohT_r = opool.tile([P, P], F32R, tag="onehotT")
                        nc.vector.tensor_tensor(
                            out=ohT_r[:],
                            in0=iota_col[:].to_broadcast([P, P]),
                            in1=row_ps[:],
                            op=mybir.AluOpType.is_equal,
                        )
                        nc.tensor.matmul(
                            out=ea_ps[:, k * HEADS : (k + 1) * HEADS],
                            lhsT=ohT_r[:],
                            rhs=adst_all[:, t * HEADS : (t + 1) * HEADS],
                            start=True,
                            stop=True,
                        )
                    # batched alpha / leaky-relu (DVE) / exp (ACT)
                    alpha_sb = spool.tile([P, K_G * HEADS], F32, tag="alpha")
                    nc.vector.tensor_add(
                        alpha_sb[:].rearrange("p (k h) -> p k h", k=K_G),
                        xa_v[:, :, HIDDEN : HIDDEN + HEADS],
                        ea_ps[:].rearrange("p (k h) -> p k h", k=K_G),
                    )
                    asc_sb = spool.tile([P, K_G * HEADS], F32, tag="asc")
                    nc.vector.tensor_scalar_mul(asc_sb[:], alpha_sb[:], NEG_SLOPE)
                    lr_sb = spool.tile([P, K_G * HEADS], F32, tag="lr")
                    nc.vector.tensor_tensor(
                        out=lr_sb[:],
                        in0=alpha_sb[:],
                        in1=asc_sb[:],
                        op=mybir.AluOpType.max,
                    )
                    ex_all = spool.tile([P, K_G * HEADS], F32R, tag="ex")
                    nc.scalar.activation(
                        out=ex_all[:],
                        in_=lr_sb[:],
                        func=mybir.ActivationFunctionType.Exp,
                    )
                    # batched msg = x * ex (per head)
                    msg_all = mpool.tile([P, K_G * HIDDEN], F32R, tag="msg")
                    nc.vector.tensor_tensor(
                        out=msg_all[:].rearrange("p (k h c) -> p k h c", k=K_G, h=HEADS),
                        in0=xa_v[:, :, :HIDDEN].rearrange(
                            "p k (h c) -> p k h c", h=HEADS
                        ),
                        in1=ex_all[:]
                        .rearrange("p (k h) -> p k h", k=K_G)
                        .unsqueeze(-1)
                        .to_broadcast([P, K_G, HEADS, CH]),
                        op=mybir.AluOpType.mult,
                    )
                    # accumulation streak on PE
                    for k in range(K_G):
                        oh_k = oh_all[:, k * P : (k + 1) * P]
                        nc.tensor.matmul(
                            out=den_ps[:],
                            lhsT=oh_k,
                            rhs=ex_all[:, k * HEADS : (k + 1) * HEADS],
                            start=(k == 0),
                            stop=(k == K_G - 1),
                        )
                        nc.tensor.matmul(
                            out=out_ps[:],
                            lhsT=oh_k,
                            rhs=msg_all[:, k * HIDDEN : (k + 1) * HIDDEN],
                            start=(k == 0),
                            stop=(k == K_G - 1),
                        )
                    # normalize: h_next = out / denom
                    den_sb = spool.tile([P, HEADS], F32, tag="den")
                    nc.vector.tensor_scalar_add(den_sb[:], den_ps[:], 1e-30)
                    dinv_sb = spool.tile([P, HEADS], F32, tag="dinv")
                    nc.vector.reciprocal(dinv_sb[:], den_sb[:])
                    nc.vector.tensor_tensor(
                        out=hout[:, hcols(t)].rearrange("p (h c) -> p h c", h=HEADS),
                        in0=out_ps[:].rearrange("p (h c) -> p h c", h=HEADS),
                        in1=dinv_sb[:].unsqueeze(-1).to_broadcast([P, HEADS, CH]),
                        op=mybir.AluOpType.mult,
                    )

                if probe.get("stop_after") == f"gat{layer + 1}":
                    return nc
            # ================= stage 5: score =================
            for t in range(T):
                prod = gpool.tile([P, HIDDEN], F32, tag="xa_sb")
                nc.vector.tensor_mul(prod[:], hB[:, hcols(t)], worep_sb[:])
                red = spool.tile([P, 1], F32, tag="red")
                nc.vector.tensor_reduce(
                    out=red[:],
                    in_=prod[:],
                    axis=mybir.AxisListType.X,
                    op=mybir.AluOpType.add,
                )
                sc = spool.tile([P, 1], F32, tag="sc")
                nc.vector.tensor_add(sc[:], red[:], scb_sb[:])
                nc.sync.dma_start(score_out[t * P : (t + 1) * P], sc[:])

    nc.compile()
    return nc


# ---------------------------------------------------------------------------
# entry point
# ---------------------------------------------------------------------------

_CACHE = {}


def prepare(inputs, plan):
    """Build (in_maps, nc, perm) from the full input dict + plan."""
    x = np.asarray(inputs["x"], np.float32)
    edge_type = np.asarray(inputs["edge_type"], np.int32)
    edge_weight = np.asarray(inputs["edge_weight"], np.float32)
    rel_emb = np.asarray(inputs["rel_emb"], np.float32)
    Wp = np.asarray(inputs["Wp"], np.float32)
    bp = np.asarray(inputs["bp"], np.float32)
    W1 = np.asarray(inputs["W1"], np.float32)
    W2 = np.asarray(inputs["W2"], np.float32)
    att_src1 = np.asarray(inputs["att_src1"], np.float32)
    att_dst1 = np.asarray(inputs["att_dst1"], np.float32)
    att_src2 = np.asarray(inputs["att_src2"], np.float32)
    att_dst2 = np.asarray(inputs["att_dst2"], np.float32)
    b1 = np.asarray(inputs["b1"], np.float32)
    b2 = np.asarray(inputs["b2"], np.float32)
    Wo = np.asarray(inputs["Wo"], np.float32)
    bo = np.asarray(inputs["bo"], np.float32)

    perm = plan["perm"]
    K_R, K_G = plan["K_R"], plan["K_G"]

    # ---- per-core dense inputs ----
    xr = np.concatenate([x[:, CODE_DIM:], CODE_WEIGHT * x[:, :CODE_DIM]], axis=1)
    xpad = np.zeros((NPAD, IN_DIM), np.float32)
    xpad[perm] = xr
    # [NCORES, T, KPROJ, P(feat), P(node)]
    xtt = (
        xpad.reshape(NCORES, T, P, KPROJ, P)
        .transpose(0, 1, 3, 4, 2)
        .copy()
    )

    w1aug = np.concatenate(
        [W1, W1 @ _asrc_mat(att_src1), W1 @ _asrc_mat(att_dst1)], axis=1
    )
    w2aug = np.concatenate(
        [W2, W2 @ _asrc_mat(att_src2), W2 @ _asrc_mat(att_dst2)], axis=1
    )
    b1w2 = (b1 @ w2aug).reshape(1, XAW).astype(np.float32)
    sc_bias = float(b2 @ Wo[:, 0] + bo[0])

    # ---- per-edge rel wtype rows: w_e * onehot5(type_e) ----
    eord_r = plan["eord_r"]  # [NTILES, P, K_R]
    wtyp = np.zeros((NTILES, P, K_R, NRELP), np.float32)
    valid = eord_r >= 0
    ew = np.where(valid, edge_weight[np.clip(eord_r, 0, E - 1)], 0.0).astype(np.float32)
    et = np.where(valid, edge_type[np.clip(eord_r, 0, E - 1)], 0)
    ii, jj, kk = np.nonzero(valid)
    wtyp[ii, jj, kk, et[ii, jj, kk]] = ew[ii, jj, kk]
    wtyp = wtyp.reshape(NTILES, P, K_R * NRELP)

    key = (K_R, K_G)
    if key not in _CACHE:
        _CACHE[key] = _build_bass(K_R, K_G)
    nc = _CACHE[key]

    common = dict(
        wp=np.ascontiguousarray(Wp.reshape(KPROJ, P, HIDDEN)),
        bp_row=bp.reshape(1, HIDDEN),
        w1aug=np.ascontiguousarray(w1aug.reshape(2, P, XAW)),
        w2aug=np.ascontiguousarray(w2aug.reshape(2, P, XAW)),
        b1w2_row=b1w2,
        rel_emb=np.concatenate([rel_emb, np.zeros((NRELP - NREL, HIDDEN), np.float32)]),
        wo_rep=np.ascontiguousarray(np.broadcast_to(Wo[:, 0], (P, HIDDEN))),
        sc_bias=np.full((P, 1), sc_bias, np.float32),
    )
    in_maps = []
    for c in range(NCORES):
        ts = slice(c * T, (c + 1) * T)
        in_maps.append(
            dict(
                common,
                xtt=xtt[c],
                esrc_r=plan["esrc_r"][ts],
                dloc_r=plan["dloc_r"][ts],
                wtyp_r=np.ascontiguousarray(wtyp[ts]),
                esrc_g=plan["esrc_g"][ts],
                dloc_g=plan["dloc_g"][ts],
            )
        )
    return in_maps, nc, perm


def kernel(x, edge_index, **rest):
    inputs = dict(rest, x=x, edge_index=edge_index)
    edge_index = np.asarray(edge_index, np.int32)
    plan = _build_plan(edge_index)
    in_maps, nc, perm = prepare(inputs, plan)

    import os

    trace = bool(os.environ.get("GAT_TRACE"))
    res = run_bass_kernel_spmd(
        nc, in_maps, core_ids=list(range(NCORES)), trace=trace
    )
    global _LAST_RESULT
    _LAST_RESULT = res
    scores_pad = np.concatenate([r["score"] for r in res.results])
    return scores_pad[perm].astype(np.float32)


_LAST_RESULT = None
